# revision 13
# baseline (speedup 1.0000x reference)
"""DoRA linear layer (nn_DoraLinearLayer) on 8 Trainium2 NeuronCores.

Math: out = (s-1)*(x @ W.T) + 2*s*((x @ A.T) @ B.T),
      s = magnitude / ||W + 2*B@A||_row  (stop-grad norm)

This factors exactly into ONE matmul per token: out = x @ Weff.T with
      n2     = ||W||²_row + Σ_r (2B).T ∘ (2A@W.T + G@(2B).T)   (G = A@A.T)
      s      = magnitude / sqrt(n2)
      Weff.T = (s-1) ∘ W.T + A.T @ (s∘(2B).T)
Device tensors carry a host-side ×16 scale (wt16, b2t16, mag16) so the
squares land in fp8e4m3 range: sq = wt16² is fp8 and ||W||² accumulates
via fp8 DoubleRow matmuls (K=256 per instruction); the 1/16 folds into
the PSUM drain (psum = 16·out, drained via tensor_scalar_mul ×1/16).
G is a 16×16 host-marshaled Gram of the rank-16 adapter.

The norm pipeline has no PE→DVE→PE round trip: h and sq depend only on
the W.T DMA (waved across both HWDGE queues), squares alternate between
the vector and scalar engines, and every setup matmul streams full
K=128 so the PE's activity-driven clock ramp starts immediately. The
weff build (one rank-16 matmul + two DVE ops per chunk) is emitted
interleaved with the chunk-major first token group so production just
trails consumption. Main loop: one fp16 matmul per token tile with
fp32 PSUM, per-tile DVE drains, per-tile output DMAs.

Sharding: column-parallel over out_features — core i owns rows
[i*512, (i+1)*512) of W/B/magnitude, x and A replicated, output shard
concatenated on the last dim on the host. Host-side work is marshaling
only: casts to fp16, transposes, slicing, static scaling, the 16×16
adapter Gram.
"""
import numpy as np

import concourse.bass as bass
import concourse.tile as tile
from concourse import bacc, mybir
from concourse.bass_utils import run_bass_kernel_spmd

N_CORES = 8
TOKENS, D_IN, D_OUT, R = 8192, 4096, 4096, 16
O = D_OUT // N_CORES          # 512 output features per core
P = 128                       # partitions
NCH = D_IN // P               # 32 contraction chunks
SCALING = 2.0                 # lora_alpha / r
SC16 = 16.0                   # static ×16 device scale (fp8 sq range)
N_WARM = 4                    # PE warmup matmuls
N_FILL = 5                    # PE filler matmuls over the s-chain

# token groups: first is chunk-major (6 PSUM banks) so the matmuls
# trail the weff feeder; the rest are tile-major; small last group
TGROUPS = [(0, 768, True)]
_t = 768
while _t + 512 <= TOKENS - 256:
    TGROUPS.append((_t, 512, False))
    _t += 512
TGROUPS.append((_t, 256, False))

f16 = mybir.dt.float16
f32 = mybir.dt.float32
bf16 = mybir.dt.bfloat16
f8 = mybir.dt.float8e4
Square = mybir.ActivationFunctionType.Square

_CACHE: dict = {}


def emit_kernel(nc, tc, xt, wt, a, a2t, g, b2t, mag, out):
    """Emit the per-core program. All DRAM APs are per-core shapes."""
    from contextlib import ExitStack
    from concourse.tile_rust import add_dep_helper

    DoubleRow = mybir.MatmulPerfMode.DoubleRow

    with ExitStack() as ctx:
        singles = ctx.enter_context(tc.tile_pool(name="singles", bufs=1))
        setup = ctx.enter_context(tc.tile_pool(name="setup", bufs=3))
        # 8 PSUM banks: gen(6: warm + main mm) + lws(1) + nh(1: n2 rows
        # 0:16, h rows 32:48, then the full-width s broadcast)
        ps_gen = ctx.enter_context(tc.tile_pool(name="ps_gen", bufs=6, space="PSUM"))
        ps_lws = ctx.enter_context(tc.tile_pool(name="ps_lws", bufs=1, space="PSUM"))
        ps_nh = ctx.enter_context(tc.tile_pool(name="ps_nh", bufs=1, space="PSUM"))
        xpool = ctx.enter_context(tc.tile_pool(name="xpool", bufs=2))
        x0pool = ctx.enter_context(tc.tile_pool(name="x0pool", bufs=1))
        opool = ctx.enter_context(tc.tile_pool(name="opool", bufs=4))

        # ---- warmup operand memsets first (smallest critical path) ----
        ones128 = singles.tile([P, P], f16)
        nc.vector.memset(ones128, 1.0)
        warm_rhs = singles.tile([P, O], f16)
        nc.vector.memset(warm_rhs, 0.002)
        ones8 = singles.tile([P, 2, 16], f8)
        nc.vector.memset(ones8, 1.0)
        ones16 = singles.tile([R, R], f16)
        nc.vector.memset(ones16, 1.0)
        ones_row32 = singles.tile([1, P], f32)
        nc.vector.memset(ones_row32, 1.0)
        sqrt_warm = singles.tile([1, 1], f32)
        nc.vector.memset(sqrt_warm, 1.0)
        sqrt_warm2 = singles.tile([1, 1], f32)
        nc.scalar.sqrt(sqrt_warm2, sqrt_warm)

        # ---- small inputs on the sync ring ahead of the W.T waves ----
        a2t_sb = singles.tile([P, NCH, R], f16)
        nc.sync.dma_start(out=a2t_sb, in_=a2t.rearrange("p (c r) -> p c r", r=R))
        a_sb = singles.tile([R, D_IN], f16)
        nc.sync.dma_start(out=a_sb, in_=a)
        b2t_sb = singles.tile([R, O], f16)
        nc.sync.dma_start(out=b2t_sb, in_=b2t)
        g_sb = singles.tile([R, R], f16)
        nc.sync.dma_start(out=g_sb, in_=g)
        mag_sb = singles.tile([1, O], f32)
        nc.gpsimd.dma_start(out=mag_sb, in_=mag)

        # ---- PE warmup: full-K fp16 matmuls start the clock ramp ----
        warm_ps = ps_gen.tile([P, O], f32, name="gen")
        for _ in range(N_WARM):
            nc.tensor.matmul(warm_ps, lhsT=ones128, rhs=warm_rhs,
                             start=True, stop=True)

        # ---- 16·W.T: host-prearranged [p, c, o]; waves alternate
        # between the sync and scalar HWDGE queues for 2x issue rate
        wt_sb = singles.tile([P, NCH, O], f16)
        wt_r = wt.rearrange("p (c o) -> p c o", o=O)
        wt_dmas = []
        wave_edges = [0, 1, 2, 3, 5, 7, 10, 13, 17, 21, 26, NCH]
        for w in range(len(wave_edges) - 1):
            lo, hi = wave_edges[w], wave_edges[w + 1]
            eng = nc.sync if w % 2 == 0 else nc.scalar
            wt_dmas.append(
                eng.dma_start(out=wt_sb[:, lo:hi, :],
                              in_=wt_r[:, lo:hi, :]))
        wt_t = [wt_sb[:, c, :] for c in range(NCH)]

        # ---- norm pipeline, trailing the wt DMA (no PE round trip):
        #   DVE/ACT: sq[c] = wt16[c]²  (fp8, engines alternate)
        #   PE: h += (2A).T_c-major @ wt16[c]   (rows 32:48 of nh bank)
        #   PE: n2 += ones @ sq-pair  (fp8 DoubleRow, rows 0:16)
        sq_sb = singles.tile([P, NCH, O], f8)
        nh = ps_nh.tile([P, O], f32)
        n2_ap = nh[0:16, :]
        h_ap = nh[32:48, :]
        LAGP = 4
        n2_next = 0

        def emit_n2(k):
            nc.tensor.matmul(n2_ap, lhsT=ones8,
                             rhs=sq_sb[:, 2 * k:2 * k + 2, :],
                             perf_mode=DoubleRow,
                             start=(k == 0), stop=False)

        for c in range(NCH):
            if c % 2 == 0:
                nc.vector.tensor_mul(sq_sb[:, c, :], wt_t[c], wt_t[c])
            else:
                nc.scalar.activation(sq_sb[:, c, :], wt_t[c], Square)
            nc.tensor.matmul(h_ap, lhsT=a2t_sb[:, c, :], rhs=wt_t[c],
                             start=(c == 0), stop=False)
            if c >= 2 * LAGP and c % 2 == 1:
                emit_n2(n2_next)
                n2_next += 1
        for k in range(n2_next, NCH // 2):
            emit_n2(k)
        # G-term folded onto the h chain, then the correction row-sum
        nc.tensor.matmul(h_ap, lhsT=g_sb, rhs=b2t_sb, start=False, stop=True)
        hterm = singles.tile([R, O], f16)
        nc.vector.tensor_mul(hterm, b2t_sb, h_ap)
        nc.tensor.matmul(n2_ap, lhsT=ones16, rhs=hterm, start=False, stop=True)

        # ---- prefetch x.T for the first two token groups; chunk DMAs
        # alternate queues, gated until W.T has fully landed on both
        t0, ntok0, _ = TGROUPS[0]
        xt0 = x0pool.tile([P, NCH, ntok0], f16, name="xt0")
        xg0 = xt[:, t0: t0 + ntok0].rearrange("(c p) t -> c p t", p=P)
        gated = {0: False, 1: False}
        for c in range(NCH):
            q = c % 2
            eng = nc.sync if q == 0 else nc.scalar
            dma = eng.dma_start(out=xt0[:, c, :], in_=xg0[c])
            if not gated[q]:
                gated[q] = True
                for wd in wt_dmas:
                    add_dep_helper(dma.ins, wd.ins, True, "x prefetch after wt")
        t1, ntok1, _ = TGROUPS[1]
        xt1 = xpool.tile([P, NCH, ntok1], f16, name="xt")
        xg1 = xt[:, t1: t1 + ntok1].rearrange("(c p) t -> c p t", p=P)
        nc.sync.dma_start(out=xt1, in_=xg1.rearrange("c p t -> p c t"))
        xt_pre = {0: xt0, 1: xt1}

        # ---- fillers keep the PE dense through the s-chain latency ----
        for _ in range(N_FILL):
            nc.tensor.matmul(warm_ps, lhsT=ones128, rhs=warm_rhs,
                             start=True, stop=True)

        # ---- s = mag16 / sqrt(n2); broadcast into the nh bank ----
        nrm = singles.tile([1, O], f32)
        nc.scalar.sqrt(nrm, nh[0:1, :])
        rn = singles.tile([1, O], f32)
        nc.vector.reciprocal_approx_fast(out=rn, in_=nrm)
        s_row = singles.tile([1, O], f32)
        nc.vector.tensor_mul(s_row, mag_sb, rn)
        nc.tensor.matmul(nh, lhsT=ones_row32, rhs=s_row, start=True, stop=True)
        sm1_sb = singles.tile([P, O], f32)
        nc.vector.tensor_scalar_add(sm1_sb, nh, -1.0)
        b2st = singles.tile([R, O], f16)
        nc.vector.tensor_mul(b2st, b2t_sb, nh[0:R, :])

        # ---- main with interleaved weff build:
        # per chunk: lws = A.T_c @ (s∘b2t16) on a single rotating bank;
        # weff16[c] = sm1∘wt16[c] + lws, written in place over wt16.
        # Group 0 is chunk-major so consumption trails this feeder.
        def emit_weff(c):
            lws = ps_lws.tile([P, O], f32, name="lws")
            nc.tensor.matmul(lws, lhsT=a_sb[:, c * P:(c + 1) * P], rhs=b2st,
                             start=True, stop=True)
            tmp = setup.tile([P, O], f32, name="tmp")
            nc.vector.tensor_mul(tmp, wt_t[c], sm1_sb)
            nc.vector.tensor_add(wt_t[c], tmp, lws)

        weff_t = wt_t
        for gi, (t0, ntok, chunk_major) in enumerate(TGROUPS):
            nm = ntok // P
            if gi in xt_pre:
                xt_t = xt_pre[gi]
            else:
                xt_t = xpool.tile([P, NCH, ntok], f16, name="xt")
                xg = xt[:, t0: t0 + ntok].rearrange("(c p) t -> c p t", p=P)
                nc.sync.dma_start(out=xt_t, in_=xg.rearrange("c p t -> p c t"))
            if chunk_major:
                emit_weff(0)
                emit_weff(1)
                pss = [ps_gen.tile([P, O], f32, name="gen") for _ in range(nm)]
                for c in range(NCH):
                    if c + 2 < NCH:
                        emit_weff(c + 2)
                    for m in range(nm):
                        nc.tensor.matmul(
                            pss[m],
                            lhsT=xt_t[:, c, m * P: (m + 1) * P],
                            rhs=weff_t[c],
                            start=(c == 0), stop=(c == NCH - 1),
                        )
                for m in range(nm):
                    ot = opool.tile([P, O], f32, name="ot")
                    nc.vector.tensor_scalar_mul(ot, pss[m], 1.0 / SC16)
                    nc.scalar.dma_start(
                        out=out[t0 + m * P: t0 + (m + 1) * P, :], in_=ot)
            else:
                for m in range(nm):
                    ps = ps_gen.tile([P, O], f32, name="gen")
                    for c in range(NCH):
                        nc.tensor.matmul(
                            ps,
                            lhsT=xt_t[:, c, m * P: (m + 1) * P],
                            rhs=weff_t[c],
                            start=(c == 0), stop=(c == NCH - 1),
                        )
                    ot = opool.tile([P, O], f32, name="ot")
                    nc.vector.tensor_scalar_mul(ot, ps, 1.0 / SC16)
                    nc.scalar.dma_start(
                        out=out[t0 + m * P: t0 + (m + 1) * P, :], in_=ot)


def build_nc():
    if "nc" in _CACHE:
        return _CACHE["nc"]
    nc = bacc.Bacc("TRN2", target_bir_lowering=False, debug=False,
                   num_devices=N_CORES)
    xt = nc.dram_tensor("xt", [D_IN, TOKENS], f16, kind="ExternalInput").ap()
    wt = nc.dram_tensor("wt", [P, NCH * O], f16, kind="ExternalInput").ap()
    a = nc.dram_tensor("a", [R, D_IN], f16, kind="ExternalInput").ap()
    a2t = nc.dram_tensor("a2t", [P, NCH * R], f16, kind="ExternalInput").ap()
    g = nc.dram_tensor("g", [R, R], f16, kind="ExternalInput").ap()
    b2t = nc.dram_tensor("b2t", [R, O], f16, kind="ExternalInput").ap()
    mag = nc.dram_tensor("mag", [1, O], f32, kind="ExternalInput").ap()
    out = nc.dram_tensor("out", [TOKENS, O], f32, kind="ExternalOutput").ap()
    with tile.TileContext(nc) as tc:
        emit_kernel(nc, tc, xt, wt, a, a2t, g, b2t, mag, out)
    nc.compile()
    _CACHE["nc"] = nc
    return nc


def prep_in_maps(x, lora_A_w, lora_B_w, base_w, magnitude):
    xt_np = np.ascontiguousarray(x.astype(np.float16).T)
    a32 = lora_A_w.astype(np.float32)
    a_np = np.ascontiguousarray(a32.astype(np.float16))
    # (2A).T partition-major: a2t[p, c*R + r] = 2·A.T[c*128 + p, r]
    a2t_full = np.ascontiguousarray((2.0 * a32).astype(np.float16).T)
    a2t_np = np.ascontiguousarray(
        a2t_full.reshape(NCH, P, R).transpose(1, 0, 2).reshape(P, NCH * R))
    g_np = np.ascontiguousarray((a32 @ a32.T).astype(np.float16))
    in_maps = []
    for c in range(N_CORES):
        sl = slice(c * O, (c + 1) * O)
        # 16·W.T partition-major: wt_dev[p, c*O + o] = 16·W.T[c*128 + p, o]
        wt_sh = np.ascontiguousarray(
            (SC16 * base_w[sl].astype(np.float32)).astype(np.float16).T)
        wt_dev = np.ascontiguousarray(
            wt_sh.reshape(NCH, P, O).transpose(1, 0, 2).reshape(P, NCH * O))
        in_maps.append({
            "xt": xt_np,
            "wt": wt_dev,
            "a": a_np,
            "a2t": a2t_np,
            "g": g_np,
            "b2t": np.ascontiguousarray(
                (SC16 * SCALING * lora_B_w[sl].astype(np.float32))
                .astype(np.float16).T),
            "mag": np.ascontiguousarray(
                (SC16 * magnitude[sl]).reshape(1, O).astype(np.float32)),
        })
    return in_maps


def kernel(x, lora_A_w, lora_B_w, base_w, magnitude):
    nc = build_nc()
    in_maps = prep_in_maps(x, lora_A_w, lora_B_w, base_w, magnitude)
    res = run_bass_kernel_spmd(nc, in_maps, list(range(N_CORES)))
    return np.concatenate(
        [res.results[c]["out"] for c in range(N_CORES)], axis=1)


# revision 19
# speedup vs baseline: 1.0544x; 1.0544x over previous
"""DoRA linear layer (nn_DoraLinearLayer) on 8 Trainium2 NeuronCores.

Math: out = (s-1)*(x @ W.T) + 2*s*((x @ A.T) @ B.T),
      s = magnitude / ||W + 2*B@A||_row  (stop-grad norm)

This factors exactly into ONE matmul per token: out = x @ Weff.T with
      n2     = ||W||²_row + Σ_r (2B).T ∘ (2A@W.T + G@(2B).T)   (G = A@A.T)
      s      = magnitude / sqrt(n2)
      Weff.T = (s-1) ∘ W.T + A.T @ (s∘(2B).T)
Device tensors carry a host-side ×16 scale (wt16, b2t16, mag16) so the
squares land in fp8e4m3 range: sq = wt16² is fp8 and ||W||² accumulates
via fp8 DoubleRow matmuls (K=256 per instruction); the 1/16 folds into
the PSUM drain (psum = 16·out, drained via tensor_scalar_mul ×1/16).
G is a 16×16 host-marshaled Gram of the rank-16 adapter.

The norm pipeline has no PE→DVE→PE round trip: h and sq depend only on
the W.T DMA (waved across both HWDGE queues), squares alternate between
the vector and scalar engines, and every setup matmul streams full
K=128 so the PE's activity-driven clock ramp starts immediately. The
weff build (one rank-16 matmul + two DVE ops per chunk) is emitted
interleaved with the chunk-major first token group so production just
trails consumption. Main loop: one fp16 matmul per token tile with
fp32 PSUM, per-tile DVE drains, per-tile output DMAs.

Sharding: column-parallel over out_features — core i owns rows
[i*512, (i+1)*512) of W/B/magnitude, x and A replicated, output shard
concatenated on the last dim on the host. Host-side work is marshaling
only: casts to fp16, transposes, slicing, static scaling, the 16×16
adapter Gram.
"""
import numpy as np

import concourse.bass as bass
import concourse.tile as tile
from concourse import bacc, mybir
from concourse.bass_utils import run_bass_kernel_spmd

N_CORES = 8
TOKENS, D_IN, D_OUT, R = 8192, 4096, 4096, 16
O = D_OUT // N_CORES          # 512 output features per core
P = 128                       # partitions
NCH = D_IN // P               # 32 contraction chunks
SCALING = 2.0                 # lora_alpha / r
SC16 = 16.0                   # static ×16 device scale (fp8 sq range)
N_WARM = 4                    # PE warmup matmuls
N_FILL = 5                    # PE filler matmuls over the s-chain

# token groups: first is chunk-major (6 PSUM banks) so the matmuls
# trail the weff feeder; the rest are tile-major; small last group
TGROUPS = [(0, 768, True)]
_t = 768
while _t + 512 <= TOKENS - 256:
    TGROUPS.append((_t, 512, False))
    _t += 512
TGROUPS.append((_t, 256, False))

f16 = mybir.dt.float16
f32 = mybir.dt.float32
bf16 = mybir.dt.bfloat16
f8 = mybir.dt.float8e4
Square = mybir.ActivationFunctionType.Square
Copy = mybir.ActivationFunctionType.Copy

_CACHE: dict = {}


def emit_kernel(nc, tc, xt, wt, a, a2t, g, b2t, mag, out):
    """Emit the per-core program. All DRAM APs are per-core shapes."""
    from contextlib import ExitStack
    from concourse.tile_rust import add_dep_helper

    DoubleRow = mybir.MatmulPerfMode.DoubleRow

    with ExitStack() as ctx:
        singles = ctx.enter_context(tc.tile_pool(name="singles", bufs=1))
        setup = ctx.enter_context(tc.tile_pool(name="setup", bufs=3))
        # 8 PSUM banks: gen(6: warm + main mm) + lws(1) + nh(1: n2 rows
        # 0:16, h rows 32:48, then the full-width s broadcast)
        ps_gen = ctx.enter_context(tc.tile_pool(name="ps_gen", bufs=6, space="PSUM"))
        ps_lws = ctx.enter_context(tc.tile_pool(name="ps_lws", bufs=1, space="PSUM"))
        ps_nh = ctx.enter_context(tc.tile_pool(name="ps_nh", bufs=1, space="PSUM"))
        xpool = ctx.enter_context(tc.tile_pool(name="xpool", bufs=2))
        x0pool = ctx.enter_context(tc.tile_pool(name="x0pool", bufs=1))
        opool = ctx.enter_context(tc.tile_pool(name="opool", bufs=4))

        # ---- warmup operand memsets first (smallest critical path) ----
        ones128 = singles.tile([P, P], f16)
        nc.vector.memset(ones128, 1.0)
        warm_rhs = singles.tile([P, O], f16)
        nc.vector.memset(warm_rhs, 0.002)
        ones8 = singles.tile([P, 2, 16], f8)
        nc.vector.memset(ones8, 1.0)
        ones16 = singles.tile([R, R], f16)
        nc.vector.memset(ones16, 1.0)
        ones_row32 = singles.tile([1, P], f32)
        nc.vector.memset(ones_row32, 1.0)
        sqrt_warm = singles.tile([1, 1], f32)
        nc.vector.memset(sqrt_warm, 1.0)
        sqrt_warm2 = singles.tile([1, 1], f32)
        nc.scalar.sqrt(sqrt_warm2, sqrt_warm)

        # ---- small inputs on the sync ring ahead of the W.T waves ----
        a2t_sb = singles.tile([P, NCH, R], f16)
        nc.sync.dma_start(out=a2t_sb, in_=a2t.rearrange("p (c r) -> p c r", r=R))
        a_sb = singles.tile([R, D_IN], f16)
        nc.sync.dma_start(out=a_sb, in_=a)
        b2t_sb = singles.tile([R, O], f16)
        nc.sync.dma_start(out=b2t_sb, in_=b2t)
        g_sb = singles.tile([R, R], f16)
        nc.sync.dma_start(out=g_sb, in_=g)
        mag_sb = singles.tile([1, O], f32)
        nc.gpsimd.dma_start(out=mag_sb, in_=mag)

        # ---- PE warmup: full-K fp16 matmuls start the clock ramp ----
        warm_ps = ps_gen.tile([P, O], f32, name="gen")
        for _ in range(N_WARM):
            nc.tensor.matmul(warm_ps, lhsT=ones128, rhs=warm_rhs,
                             start=True, stop=True)

        # ---- 16·W.T: host-prearranged [p, c, o]; waves alternate
        # between the sync and scalar HWDGE queues for 2x issue rate
        wt_sb = singles.tile([P, NCH, O], f16)
        wt_r = wt.rearrange("p (c o) -> p c o", o=O)
        wt_dmas = []
        wave_edges = [0, 1, 2, 4, 7, 11, 16, 23, NCH]
        for w in range(len(wave_edges) - 1):
            lo, hi = wave_edges[w], wave_edges[w + 1]
            wt_dmas.append(
                nc.sync.dma_start(out=wt_sb[:, lo:hi, :],
                                  in_=wt_r[:, lo:hi, :]))
        wt_t = [wt_sb[:, c, :] for c in range(NCH)]

        # ---- norm pipeline, trailing the wt DMA (no PE round trip):
        #   DVE/ACT: sq[c] = wt16[c]²  (fp8, engines alternate)
        #   PE: h += (2A).T_c-major @ wt16[c]   (rows 32:48 of nh bank)
        #   PE: n2 += ones @ sq-pair  (fp8 DoubleRow, rows 0:16)
        sq_sb = singles.tile([P, NCH, O], f8)
        nh = ps_nh.tile([P, O], f32)
        n2_ap = nh[0:16, :]
        h_ap = nh[32:48, :]
        LAGP = 4
        n2_next = 0

        def emit_n2(k):
            nc.tensor.matmul(n2_ap, lhsT=ones8,
                             rhs=sq_sb[:, 2 * k:2 * k + 2, :],
                             perf_mode=DoubleRow,
                             start=(k == 0), stop=False)

        for c in range(NCH):
            if c % 2 == 0:
                nc.vector.tensor_mul(sq_sb[:, c, :], wt_t[c], wt_t[c])
            else:
                nc.scalar.activation(sq_sb[:, c, :], wt_t[c], Square)
            nc.tensor.matmul(h_ap, lhsT=a2t_sb[:, c, :], rhs=wt_t[c],
                             start=(c == 0), stop=False)
            if c >= 2 * LAGP and c % 2 == 1:
                emit_n2(n2_next)
                n2_next += 1
        for k in range(n2_next, NCH // 2):
            emit_n2(k)
        # G-term folded onto the h chain, then the correction row-sum
        nc.tensor.matmul(h_ap, lhsT=g_sb, rhs=b2t_sb, start=False, stop=True)
        hterm = singles.tile([R, O], f16)
        nc.vector.tensor_mul(hterm, b2t_sb, h_ap)
        nc.tensor.matmul(n2_ap, lhsT=ones16, rhs=hterm, start=False, stop=True)

        # ---- fillers keep the PE dense through the s-chain latency ----
        for _ in range(N_FILL):
            nc.tensor.matmul(warm_ps, lhsT=ones128, rhs=warm_rhs,
                             start=True, stop=True)

        # ---- s = mag16 / sqrt(n2); broadcast into the nh bank ----
        nrm = singles.tile([1, O], f32)
        nc.scalar.sqrt(nrm, nh[0:1, :])
        rn = singles.tile([1, O], f32)
        nc.vector.reciprocal_approx_fast(out=rn, in_=nrm)
        s_row = singles.tile([1, O], f32)
        nc.vector.tensor_mul(s_row, mag_sb, rn)
        nc.tensor.matmul(nh, lhsT=ones_row32, rhs=s_row, start=True, stop=True)
        b2st = singles.tile([R, O], f16)
        nc.vector.tensor_mul(b2st, b2t_sb, nh[0:R, :])
        sm1_sb = singles.tile([P, O], f32)
        nc.vector.tensor_scalar_add(sm1_sb, nh, -1.0)

        # ---- prefetch x.T for the first two token groups (emitted after
        # the s-chain: DMA issue occupies the queue engine ~0.7us each,
        # and nothing in the s-chain may sit behind them); gated until
        # W.T has fully landed
        t0, ntok0, _ = TGROUPS[0]
        xt0 = x0pool.tile([P, NCH, ntok0], f16, name="xt0")
        xg0 = xt[:, t0: t0 + ntok0].rearrange("(c p) t -> c p t", p=P)
        for c in range(NCH):
            dma = nc.sync.dma_start(out=xt0[:, c, :], in_=xg0[c])
            if c == 0:
                for wd in wt_dmas:
                    add_dep_helper(dma.ins, wd.ins, True, "x prefetch after wt")
        t1, ntok1, _ = TGROUPS[1]
        xt1 = xpool.tile([P, NCH, ntok1], f16, name="xt")
        xg1 = xt[:, t1: t1 + ntok1].rearrange("(c p) t -> c p t", p=P)
        nc.sync.dma_start(out=xt1, in_=xg1.rearrange("c p t -> p c t"))
        xt_pre = {0: xt0, 1: xt1}

        # ---- main with interleaved weff build:
        # per chunk: lws = A.T_c @ (s∘b2t16) on a single rotating bank;
        # weff16[c] = sm1∘wt16[c] + lws, written in place over wt16.
        # Group 0 is chunk-major so consumption trails this feeder.
        def emit_weff(c):
            lws = ps_lws.tile([P, O], f32, name="lws")
            nc.tensor.matmul(lws, lhsT=a_sb[:, c * P:(c + 1) * P], rhs=b2st,
                             start=True, stop=True)
            tmp = setup.tile([P, O], f32, name="tmp")
            nc.vector.tensor_mul(tmp, wt_t[c], sm1_sb)
            nc.vector.tensor_add(wt_t[c], tmp, lws)

        weff_t = wt_t
        for gi, (t0, ntok, chunk_major) in enumerate(TGROUPS):
            nm = ntok // P
            if gi in xt_pre:
                xt_t = xt_pre[gi]
            else:
                xt_t = xpool.tile([P, NCH, ntok], f16, name="xt")
                xg = xt[:, t0: t0 + ntok].rearrange("(c p) t -> c p t", p=P)
                nc.sync.dma_start(out=xt_t, in_=xg.rearrange("c p t -> p c t"))
            if chunk_major:
                emit_weff(0)
                emit_weff(1)
                pss = [ps_gen.tile([P, O], f32, name="gen") for _ in range(nm)]
                for c in range(NCH):
                    if c + 2 < NCH:
                        emit_weff(c + 2)
                    for m in range(nm):
                        nc.tensor.matmul(
                            pss[m],
                            lhsT=xt_t[:, c, m * P: (m + 1) * P],
                            rhs=weff_t[c],
                            start=(c == 0), stop=(c == NCH - 1),
                        )
                for m in range(nm):
                    ot = opool.tile([P, O], f32, name="ot")
                    nc.scalar.activation(ot, pss[m], Copy, scale=1.0 / SC16)
                    nc.scalar.dma_start(
                        out=out[t0 + m * P: t0 + (m + 1) * P, :], in_=ot)
            else:
                for m in range(nm):
                    ps = ps_gen.tile([P, O], f32, name="gen")
                    for c in range(NCH):
                        nc.tensor.matmul(
                            ps,
                            lhsT=xt_t[:, c, m * P: (m + 1) * P],
                            rhs=weff_t[c],
                            start=(c == 0), stop=(c == NCH - 1),
                        )
                    ot = opool.tile([P, O], f32, name="ot")
                    nc.scalar.activation(ot, ps, Copy, scale=1.0 / SC16)
                    nc.scalar.dma_start(
                        out=out[t0 + m * P: t0 + (m + 1) * P, :], in_=ot)


def build_nc():
    if "nc" in _CACHE:
        return _CACHE["nc"]
    nc = bacc.Bacc("TRN2", target_bir_lowering=False, debug=False,
                   num_devices=N_CORES)
    xt = nc.dram_tensor("xt", [D_IN, TOKENS], f16, kind="ExternalInput").ap()
    wt = nc.dram_tensor("wt", [P, NCH * O], f16, kind="ExternalInput").ap()
    a = nc.dram_tensor("a", [R, D_IN], f16, kind="ExternalInput").ap()
    a2t = nc.dram_tensor("a2t", [P, NCH * R], f16, kind="ExternalInput").ap()
    g = nc.dram_tensor("g", [R, R], f16, kind="ExternalInput").ap()
    b2t = nc.dram_tensor("b2t", [R, O], f16, kind="ExternalInput").ap()
    mag = nc.dram_tensor("mag", [1, O], f32, kind="ExternalInput").ap()
    out = nc.dram_tensor("out", [TOKENS, O], f32, kind="ExternalOutput").ap()
    with tile.TileContext(nc) as tc:
        emit_kernel(nc, tc, xt, wt, a, a2t, g, b2t, mag, out)
    nc.compile()
    _CACHE["nc"] = nc
    return nc


def prep_in_maps(x, lora_A_w, lora_B_w, base_w, magnitude):
    xt_np = np.ascontiguousarray(x.astype(np.float16).T)
    a32 = lora_A_w.astype(np.float32)
    a_np = np.ascontiguousarray(a32.astype(np.float16))
    # (2A).T partition-major: a2t[p, c*R + r] = 2·A.T[c*128 + p, r]
    a2t_full = np.ascontiguousarray((2.0 * a32).astype(np.float16).T)
    a2t_np = np.ascontiguousarray(
        a2t_full.reshape(NCH, P, R).transpose(1, 0, 2).reshape(P, NCH * R))
    g_np = np.ascontiguousarray((a32 @ a32.T).astype(np.float16))
    in_maps = []
    for c in range(N_CORES):
        sl = slice(c * O, (c + 1) * O)
        # 16·W.T partition-major: wt_dev[p, c*O + o] = 16·W.T[c*128 + p, o]
        wt_sh = np.ascontiguousarray(
            (SC16 * base_w[sl].astype(np.float32)).astype(np.float16).T)
        wt_dev = np.ascontiguousarray(
            wt_sh.reshape(NCH, P, O).transpose(1, 0, 2).reshape(P, NCH * O))
        in_maps.append({
            "xt": xt_np,
            "wt": wt_dev,
            "a": a_np,
            "a2t": a2t_np,
            "g": g_np,
            "b2t": np.ascontiguousarray(
                (SC16 * SCALING * lora_B_w[sl].astype(np.float32))
                .astype(np.float16).T),
            "mag": np.ascontiguousarray(
                (SC16 * magnitude[sl]).reshape(1, O).astype(np.float32)),
        })
    return in_maps


def kernel(x, lora_A_w, lora_B_w, base_w, magnitude):
    nc = build_nc()
    in_maps = prep_in_maps(x, lora_A_w, lora_B_w, base_w, magnitude)
    res = run_bass_kernel_spmd(nc, in_maps, list(range(N_CORES)))
    return np.concatenate(
        [res.results[c]["out"] for c in range(N_CORES)], axis=1)
